# revision 28
# baseline (speedup 1.0000x reference)
"""Trainium2 Bass kernel for nn_MultiHeadAttention_81363860455568.

Reference computation (B=2, S=2048, D=1024, H=16, DK=64):
    qh = split_heads(q @ Wq.T); kh, vh likewise
    scores = softmax(qh @ kh.T / 8, axis=-1)
    scores = scores * reaches[:,None,None,:]            (per key)
    scores = scores * (1 - 0.999999*eye(S))             (diagonal suppression)
    out = vh - scores @ vh
    out = out * contrib[:,None,:,None]                  (per query)
    y = concat_heads(out) @ Wo.T

Sharding: 8 cores = 2 batches x 4 head-groups (4 heads each). Each core
receives its batch's transposed activations qT/kT/vT [D, S] in bf16 plus the
head-group slices of Wq/Wk/Wv (as [D, 256]) and Wo (as [256, D]), and returns
a partial y [S, D] (fp32) that the host sums across the 4 head-groups.

On-chip layout (per core, all matmuls bf16 with fp32 PSUM):
  - qhT/khT "pair" buffers [128, S]: heads (2p, 2p+1) stacked on partitions.
  - scoresT[k, q] via row-packed (K=64) matmul pairs; exp on ACT (scale=1/8).
  - diagonal handling: d2 = e*eye999 is subtracted from e in-place; the
    denominator matmul adds d2 back so softmax normalization sees unmasked e.
  - denominators: ones-vector matmul (col-packed M=1 tiles); AV: col-packed
    M=64 tiles with lhsT = reaches-scaled V in natural [k, d] layout.
  - epilogue: concatT = vhT - (AVT * 1/denom); contrib applied on the Wo
    output as a per-partition scalar.
"""

import functools

import numpy as np
import ml_dtypes

import concourse.bass as bass
import concourse.mybir as mybir
import concourse.tile as tile
from concourse import bacc
from concourse.bass_utils import run_bass_kernel_spmd
from concourse.masks import make_identity

BF16 = mybir.dt.bfloat16
F32 = mybir.dt.float32

B, S, D, H = 2, 2048, 1024, 16
DK = D // H          # 64
HG = 4               # heads per core (head group)
GD = HG * DK         # 256 head-group dims per core
NKC = D // 128       # 8 contraction chunks for projections
NKB = S // 128       # 16 key blocks
NMS = S // 128       # 16 query/row blocks
NQC = S // 512       # 4 query chunks of 512


def _emit_kernel(tc: tile.TileContext):
    nc = tc.nc

    qT = nc.declare_dram_parameter("qT", [D, S], BF16, isOutput=False).ap()
    kT = nc.declare_dram_parameter("kT", [D, S], BF16, isOutput=False).ap()
    vT = nc.declare_dram_parameter("vT", [D, S], BF16, isOutput=False).ap()
    wq = nc.declare_dram_parameter("wq", [D, GD], BF16, isOutput=False).ap()
    wk = nc.declare_dram_parameter("wk", [D, GD], BF16, isOutput=False).ap()
    wv = nc.declare_dram_parameter("wv", [D, GD], BF16, isOutput=False).ap()
    wo = nc.declare_dram_parameter("wo", [GD, D], BF16, isOutput=False).ap()
    rcol = nc.declare_dram_parameter("rcol", [128, NKB], F32, isOutput=False).ap()
    ccol = nc.declare_dram_parameter("ccol", [128, NMS], F32, isOutput=False).ap()
    y = nc.declare_dram_parameter("y", [S, D], F32, isOutput=True).ap()

    Exp = mybir.ActivationFunctionType.Exp

    # ---------------- resident SBUF buffers ----------------
    consts = tc.alloc_tile_pool(name="consts", bufs=1)
    wq_sb = consts.tile([128, NKC, GD], BF16)
    wk_sb = consts.tile([128, NKC, GD], BF16)
    wv_sb = consts.tile([128, NKC, GD], BF16)
    wo_sb = consts.tile([128, 2, D], BF16)
    rr = consts.tile([128, NKB], F32)
    cc = consts.tile([128, NMS], F32)
    eye999 = consts.tile([128, 128], F32)
    ident = consts.tile([128, 128], BF16)
    ones1 = consts.tile([128, 1], BF16)
    ones_row = consts.tile([1, 128], F32)

    res = tc.alloc_tile_pool(name="res", bufs=1)
    qhT2 = [res.tile([128, S], BF16, name=f"qhT2_{p}") for p in range(2)]
    khT2 = [res.tile([128, S], BF16, name=f"khT2_{p}") for p in range(2)]
    vhT2 = [res.tile([128, S], BF16, name=f"vhT2_{p}") for p in range(2)]
    vnat = res.tile([128, NKB, GD], BF16)   # reaches-scaled V, natural [k, d]
    catT = [res.tile([128, S], BF16, name=f"catT_{p}") for p in range(2)]
    consts.seal()
    res.seal()

    # constant setup
    nc.gpsimd.memset(ones1, 1.0)
    nc.gpsimd.memset(ones_row, 1.0)
    make_identity(nc, ident)
    nc.gpsimd.memset(eye999, 0.0)
    nc.gpsimd.affine_select(
        out=eye999, in_=eye999,
        compare_op=mybir.AluOpType.not_equal,
        fill=0.999999, base=0, pattern=[[-1, 128]], channel_multiplier=1,
    )

    # constant/weight DMAs — V-projection operands first so PE starts early
    for kc in range(NKC):
        nc.sync.dma_start(out=wv_sb[:, kc, :], in_=wv[kc * 128:(kc + 1) * 128, :])
    nc.sync.dma_start(out=rr, in_=rcol)

    # ---------------- projection phase ----------------
    # spsum is allocated OUTSIDE the projection pools so the scores matmuls
    # of the attention phase get PSUM banks disjoint from the projection
    # banks and can start before the projection PSUM pipeline drains.
    spsum_cm = tc.tile_pool(name="spsum", bufs=2, space="PSUM")
    spsum = spsum_cm.__enter__()
    with (
        tc.tile_pool(name="xres", bufs=1) as xres,
        tc.tile_pool(name="ppsum", bufs=2, space="PSUM") as ppsum,
        tc.tile_pool(name="tpsum", bufs=1, space="PSUM") as tpsum,
        tc.tile_pool(name="vtmp_pool", bufs=2) as vtmp_pool,
    ):
        vT_sb = xres.tile([128, NKC, S], BF16)
        qT_sb = xres.tile([128, NKC, S], BF16)
        kT_sb = xres.tile([128, NKC, S], BF16)
        for kc in range(NKC):
            nc.sync.dma_start(out=vT_sb[:, kc, :], in_=vT[kc * 128:(kc + 1) * 128, :])
        for kc in range(NKC):
            nc.sync.dma_start(out=wq_sb[:, kc, :], in_=wq[kc * 128:(kc + 1) * 128, :])
            nc.sync.dma_start(out=wk_sb[:, kc, :], in_=wk[kc * 128:(kc + 1) * 128, :])
        for kc in range(NKC):
            nc.sync.dma_start(out=qT_sb[:, kc, :], in_=qT[kc * 128:(kc + 1) * 128, :])
            nc.sync.dma_start(out=kT_sb[:, kc, :], in_=kT[kc * 128:(kc + 1) * 128, :])
        for p in range(2):
            nc.sync.dma_start(out=wo_sb[:, p, :], in_=wo[p * 128:(p + 1) * 128, :])
        nc.sync.dma_start(out=cc, in_=ccol)

        # V projection: vnat[k, d] (reaches-scaled) + vhT (unscaled, transposed)
        for ms in range(NMS):
            ps = ppsum.tile([128, 512], F32, tag="pp")
            for kc in range(NKC):
                nc.tensor.matmul(
                    ps[:, :GD],
                    lhsT=vT_sb[:, kc, ms * 128:(ms + 1) * 128],
                    rhs=wv_sb[:, kc, :],
                    start=(kc == 0), stop=(kc == NKC - 1),
                )
            nc.vector.tensor_scalar_mul(vnat[:, ms, :], ps[:, :GD], rr[:, ms:ms + 1])
            vtmp = vtmp_pool.tile([128, GD], BF16, tag="vtmp")
            nc.vector.tensor_copy(vtmp, ps[:, :GD])
            for p in range(2):
                tp = tpsum.tile([128, 128], BF16, tag="tp")
                nc.tensor.transpose(tp, vtmp[:, p * 128:(p + 1) * 128], ident)
                nc.vector.tensor_copy(vhT2[p][:, ms * 128:(ms + 1) * 128], tp)

        # Q/K projections into pair-stacked transposed layout
        for p in range(2):
            for (w_sb, dst) in ((wq_sb, qhT2), (wk_sb, khT2)):
                for nq in range(NQC):
                    ps = ppsum.tile([128, 512], F32, tag="pp")
                    for kc in range(NKC):
                        nc.tensor.matmul(
                            ps,
                            lhsT=w_sb[:, kc, p * 128:(p + 1) * 128],
                            rhs=(qT_sb if dst is qhT2 else kT_sb)[
                                :, kc, nq * 512:(nq + 1) * 512],
                            start=(kc == 0), stop=(kc == NKC - 1),
                        )
                    nc.vector.tensor_copy(dst[p][:, nq * 512:(nq + 1) * 512], ps)

        # Semaphore-clock warm-up: walrus caps per-instruction sync waits, and
        # the first ACT/DVE instruction after the pool boundary would
        # otherwise join all 8 HW-DMA queue sems at once. Touch each input
        # DMA (and the gpsimd consts) with tiny reads so ACT/DVE observe
        # those sems a few at a time here, where they are idle anyway.
        wscA = vtmp_pool.tile([1, 24], BF16, tag="warmA")
        wscV = vtmp_pool.tile([1, 24], BF16, tag="warmV")
        for i in range(NKC):
            for j, src in enumerate((qT_sb, kT_sb, vT_sb)):
                nc.scalar.copy(wscA[:, 3 * i + j:3 * i + j + 1], src[0:1, i, 0:1])
                nc.vector.tensor_copy(wscV[:, 3 * i + j:3 * i + j + 1], src[0:1, i, 0:1])
        wscA2 = vtmp_pool.tile([1, 8], F32, tag="warmA2")
        wscV2 = vtmp_pool.tile([1, 8], F32, tag="warmV2")
        for j, src in enumerate((rr, cc, eye999, ones_row)):
            nc.scalar.copy(wscA2[:, j:j + 1], src[0:1, 0:1])
            nc.vector.tensor_copy(wscV2[:, j:j + 1], src[0:1, 0:1])

    # ---------------- attention + output phase ----------------
    with (
        tc.tile_pool(name="apsum", bufs=1, space="PSUM") as apsum,
        tc.tile_pool(name="dwops", bufs=1, space="PSUM") as dwops,
        tc.tile_pool(name="epool", bufs=54) as epool,
        tc.tile_pool(name="d2pool", bufs=12) as d2pool,
        tc.tile_pool(name="mpool", bufs=4) as mpool,
        tc.tile_pool(name="ypool", bufs=3) as ypool,
    ):
        def emit_wo(mb):
            for oc in range(2):
                wop = dwops.tile([128, 512], F32, tag="dwo", name="wop")
                for p in range(2):
                    nc.tensor.matmul(
                        wop,
                        lhsT=catT[p][:, mb * 128:(mb + 1) * 128],
                        rhs=wo_sb[:, p, oc * 512:(oc + 1) * 512],
                        start=(p == 0), stop=(p == 1),
                    )
                y_sb = ypool.tile([128, 512], F32, tag="ysb")
                nc.vector.tensor_scalar_mul(y_sb, wop, cc[:, mb:mb + 1])
                nc.sync.dma_start(
                    out=y[mb * 128:(mb + 1) * 128, oc * 512:(oc + 1) * 512],
                    in_=y_sb,
                )

        for half in range(2):
            q0 = half * 1024
            for p in range(2):
                # ---- B1: scoresT -> exp -> diag -> denominator rows ----
                # The denominator matmuls ride along in B1 where PE is idle
                # under the ACT-bound exp stream.
                etiles = {}   # (head_local, kb) -> [128, 1024] bf16
                dp = dwops.tile([128, 1024], F32, tag="dph")
                for kb in range(NKB):
                    spair = []
                    for h in range(2):
                        sp = spsum.tile([128, 1024], F32, tag="sc")
                        r0, r1 = h * 64, h * 64 + 64
                        for qc in range(2):
                            nc.tensor.matmul(
                                sp[:, qc * 512:(qc + 1) * 512],
                                lhsT=khT2[p][r0:r1, kb * 128:(kb + 1) * 128],
                                rhs=qhT2[p][r0:r1, q0 + qc * 512:q0 + (qc + 1) * 512],
                                start=True, stop=True,
                                tile_position=(h * 64, 0),
                            )
                        spair.append(sp)
                    diag = 8 * half <= kb < 8 * half + 8
                    off = 128 * (kb - 8 * half)
                    d2s = {}
                    for h in range(2):
                        et = epool.tile([128, 1024], BF16, tag="e")
                        nc.scalar.activation(et, spair[h], Exp, scale=0.125)
                        etiles[(h, kb)] = et
                        if diag:
                            d2 = d2pool.tile([128, 128], BF16, tag="d2")
                            nc.vector.tensor_mul(
                                d2, et[:, off:off + 128], eye999)
                            nc.vector.tensor_sub(
                                et[:, off:off + 128], et[:, off:off + 128], d2)
                            d2s[h] = d2
                    # denominator rows (M=1 col-packed at col 0 / 32) over the
                    # masked e; the diagonal contribution is added back from
                    # d2 so normalization sees the unmasked sum. The add-back
                    # must land after kb 0's start=True reset and before
                    # kb 15's stop=True close.
                    for h in range(2):
                        def addback():
                            nc.tensor.matmul(
                                dp[h * 32:h * 32 + 1, off:off + 128],
                                lhsT=ones1,
                                rhs=d2s[h],
                                start=False, stop=False,
                                tile_position=(0, h * 32),
                                skip_group_check=True,
                            )
                        if diag and kb > 0:
                            addback()
                        for qc in range(2):
                            nc.tensor.matmul(
                                dp[h * 32:h * 32 + 1, qc * 512:(qc + 1) * 512],
                                lhsT=ones1,
                                rhs=etiles[(h, kb)][:, qc * 512:(qc + 1) * 512],
                                start=(kb == 0), stop=(kb == NKB - 1),
                                tile_position=(0, h * 32),
                                skip_group_check=True,
                            )
                        if diag and kb == 0:
                            addback()

                # ---- B2: AV + epilogue per 512-wide q chunk ----
                for qc in range(2):
                    wq0 = qc * 512
                    av = apsum.tile([128, 512], F32, tag="av")
                    for kb in range(NKB):
                        for h in range(2):
                            nc.tensor.matmul(
                                av[h * 64:h * 64 + 64, :],
                                lhsT=vnat[:, kb, p * 128 + h * 64:p * 128 + h * 64 + 64],
                                rhs=etiles[(h, kb)][:, wq0:wq0 + 512],
                                start=(kb == 0), stop=(kb == NKB - 1),
                                tile_position=(0, h * 64),
                                skip_group_check=True,
                            )
                    # coefficients: 1/denom broadcast across partitions via PE
                    bc = dwops.tile([128, 512], F32, tag="dwo", name="bc")
                    for h in range(2):
                        c2 = mpool.tile([1, 512], F32, tag="c2", name=f"c2_{h}")
                        nc.vector.reciprocal(c2, dp[h * 32:h * 32 + 1, wq0:wq0 + 512])
                        nc.tensor.matmul(
                            bc[h * 64:h * 64 + 64, :],
                            lhsT=ones_row[0:1, 0:64],
                            rhs=c2,
                            start=True, stop=True,
                            tile_position=(0, h * 64),
                            skip_group_check=True,
                        )
                    bcC = mpool.tile([128, 512], F32, tag="bc")
                    nc.vector.tensor_copy(bcC, bc)
                    # epilogue: catT = vhT - av * (1/denom)
                    t1 = mpool.tile([128, 512], BF16, tag="t1")
                    nc.vector.tensor_mul(t1, av, bcC)
                    nc.vector.tensor_sub(
                        catT[p][:, q0 + wq0:q0 + wq0 + 512],
                        vhT2[p][:, q0 + wq0:q0 + wq0 + 512],
                        t1,
                    )
                    # Wo for this q chunk once both pairs' epilogues are done
                    if p == 1:
                        for m in range(4):
                            emit_wo(8 * half + 4 * qc + m)
    spsum_cm.__exit__(None, None, None)


@functools.cache
def build_nc() -> bass.Bass:
    nc = bacc.Bacc("TRN2", target_bir_lowering=False, debug=False)
    with tile.TileContext(nc) as tc:
        _emit_kernel(tc)
    nc.compile()
    return nc


def _prep_inputs(q, k, v, reaches, Wq, Wk, Wv, Wo):
    """Host-side shard + layout prep. Returns per-core input maps."""
    bf16 = ml_dtypes.bfloat16
    r = np.asarray(reaches, np.float32)
    rs = r.sum(axis=-1, keepdims=True)
    contrib = (rs - r) / (rs + 1e-9) * (1.0 - r) * 100.0  # [B, S] f32

    per_batch = []
    for b in range(B):
        qTb = np.ascontiguousarray(np.asarray(q[b], np.float32).T.astype(bf16))
        kTb = np.ascontiguousarray(np.asarray(k[b], np.float32).T.astype(bf16))
        vTb = np.ascontiguousarray(np.asarray(v[b], np.float32).T.astype(bf16))
        # [128, NKB] with [p, c] = vec[128*c + p]
        rcol = np.ascontiguousarray(r[b].reshape(NKB, 128).T)
        ccol = np.ascontiguousarray(contrib[b].reshape(NMS, 128).T)
        per_batch.append((qTb, kTb, vTb, rcol, ccol))

    in_maps = []
    for c in range(8):
        b, g = divmod(c, 4)
        hs = slice(g * GD, (g + 1) * GD)
        qTb, kTb, vTb, rcol, ccol = per_batch[b]
        in_maps.append({
            "qT": qTb, "kT": kTb, "vT": vTb,
            "wq": np.ascontiguousarray(np.asarray(Wq, np.float32)[hs, :].T).astype(bf16),
            "wk": np.ascontiguousarray(np.asarray(Wk, np.float32)[hs, :].T).astype(bf16),
            "wv": np.ascontiguousarray(np.asarray(Wv, np.float32)[hs, :].T).astype(bf16),
            "wo": np.ascontiguousarray(np.asarray(Wo, np.float32)[:, hs].T).astype(bf16),
            "rcol": rcol, "ccol": ccol,
        })
    return in_maps


def kernel(q, k, v, reaches, Wq, Wk, Wv, Wo, **run_kwargs):
    nc = build_nc()
    in_maps = _prep_inputs(q, k, v, reaches, Wq, Wk, Wv, Wo)
    res = run_bass_kernel_spmd(nc, in_maps, list(range(8)), **run_kwargs)
    out = np.zeros((B, S, D), np.float32)
    for c in range(8):
        b = c // 4
        out[b] += res.results[c]["y"]
    if run_kwargs:
        kernel.last_results = res
    return out
